# revision 1
# baseline (speedup 1.0000x reference)
"""CrossAttention TRN2 kernel (v2: transposed-PV with self-interleaved
accumulation).

Full-input contract: kernel(**inputs) takes the unsharded numpy inputs of
  reference.py (q,k,v [2,2048,1024] fp32; Wq/Wk/Wv/Wo [1024,1024]; biases)
and returns the full [2,2048,1024] fp32 output.

Sharding: 8 cores = 2 batch groups x 4 head groups (tensor parallel over
heads).  Core c handles batch c//4 and heads [4*(c%4), 4*(c%4)+4).
Each core computes its heads' Q/K/V projections, attention, and a partial
output projection (row-slice of Wo); the host sums the 4 partials per batch
(no on-device collectives needed).

Per-core dataflow (all matmuls bf16 with fp32 PSUM accumulation):
  - host pre-transposes/casts activations (q^T,k^T,v^T [cin, tok] bf16) and
    weight slices, so contraction dims land on SBUF partitions directly.
  - scores are computed transposed ([ts, tq]): stationary k-tile [64,128],
    moving q [64,512].  exp on ScalarE (scale 1/8 folded), FD=1024.
  - PV runs TRANSPOSED: out^T[tq,d] accumulates over ts tiles with the exp
    tile as stationary [128,128] and v[ts,d] as moving [128,64] -- half the
    PE rows of the [65,512]-oriented PV.  A parallel 1-wide matmul against a
    ones vector accumulates the softmax denominator per tq ON PARTITIONS,
    so normalization is a per-partition reciprocal + tensor_scalar (no DMA
    broadcast round-trip).  PV consumes each exp tile the step after it is
    produced (self-interleaved), so no bulk PV drain remains at the end.
  - normalized att tiles [tq,256] are PE-transposed (identity matmul) back
    to [cout,tq] for the output projection, which is unchanged.
  - pair order (tb,hp) = (0,0),(1,0),(0,1),(1,1); drains are deferred into
    the next pair's phase-1 steps; the last pair's drain splits its PSUM
    evacuation copies between DVE and the (then idle) ScalarE.
  - q/k/v bias adds run on Pool (gpsimd) to keep DVE under the ScalarE
    roofline; ScalarE exp (~133 us) is the modeled bottleneck.
"""

import os
import numpy as np
import ml_dtypes

BF16 = ml_dtypes.bfloat16

B, TOKENS, C = 2, 2048, 1024
NHEAD, D = 16, 64
NCORES = 8
NGROUP = 4                # head groups (cores per batch)
COUT = C // NGROUP        # 256 head-channels per core
NH = NHEAD // NGROUP      # 4 heads per core

P = 128                   # SBUF partitions


def build_nc(tok=TOKENS, cin=C, cout=COUT, nh=NH, reps=1):
    """Emit the per-core Bass module. d=64 fixed; cout = nh*64.
    reps>1 replicates the whole body (timing builds only): per-rep marginal
    time = steady-state kernel time with per-exec dispatch overhead removed."""
    import concourse.bacc as bacc
    import concourse.tile as tile
    import concourse.mybir as mybir
    from concourse import masks

    d = D
    assert cout == nh * d
    ncin = cin // P               # cin tiles (contraction)
    nt = tok // P                 # ts tiles
    nm = max(1, cout // P)        # 128-wide cout chunks (qhT/khT)
    tqb = min(1024, tok)          # tq block (exp FD)
    ntqb = tok // tqb
    ntb = tqb // P                # tq tiles per block (8)
    sck = min(512, tok)           # scores moving chunk
    csk = tqb // sck
    nko = max(1, cout // P)       # out-proj contraction tiles
    nob = max(1, cin // 256)      # out-proj 256-wide chunks

    fp32 = mybir.dt.float32
    bf16 = mybir.dt.bfloat16

    nc = bacc.Bacc("TRN2", target_bir_lowering=False, debug=False)

    qT = nc.dram_tensor("qT", [cin, tok], bf16, kind="ExternalInput")
    kT = nc.dram_tensor("kT", [cin, tok], bf16, kind="ExternalInput")
    vT = nc.dram_tensor("vT", [cin, tok], bf16, kind="ExternalInput")
    wqT = nc.dram_tensor("wqT", [cin, cout], bf16, kind="ExternalInput")
    wkT = nc.dram_tensor("wkT", [cin, cout], bf16, kind="ExternalInput")
    wvT = nc.dram_tensor("wvT", [cin, cout], bf16, kind="ExternalInput")
    woT = nc.dram_tensor("woT", [cout, cin], bf16, kind="ExternalInput")
    bqv = nc.dram_tensor("bqv", [P, nm], fp32, kind="ExternalInput")
    bkv = nc.dram_tensor("bkv", [P, nm], fp32, kind="ExternalInput")
    bvv = nc.dram_tensor("bvv", [1, cout], fp32, kind="ExternalInput")
    outp = nc.dram_tensor("outp", [tok, cin], bf16, kind="ExternalOutput")

    with tile.TileContext(nc) as tc:
        from contextlib import ExitStack
        with ExitStack() as ctx:
            consts = ctx.enter_context(tc.tile_pool(name="consts", bufs=1))
            xstream = ctx.enter_context(tc.tile_pool(name="xstream", bufs=2))
            vstream = ctx.enter_context(tc.tile_pool(name="vstream", bufs=2))
            persist = ctx.enter_context(tc.tile_pool(name="persist", bufs=1))
            expool = ctx.enter_context(tc.tile_pool(name="expool", bufs=4))
            smalls = ctx.enter_context(tc.tile_pool(name="smalls", bufs=8))
            ostage = ctx.enter_context(tc.tile_pool(name="ostage", bufs=3))
            psum = ctx.enter_context(
                tc.tile_pool(name="psum", bufs=1, space="PSUM"))

            for _rep in range(reps):
                # ---- constants (K/Q m=0 weights first — gate the first exp)
                wq_sb = consts.tile([P, ncin, cout], bf16, tag="wq")
                wk_sb = consts.tile([P, ncin, cout], bf16, tag="wk")
                wv_sb = consts.tile([P, ncin, cout], bf16, tag="wv")
                wo_sb = consts.tile([P, nko, cin], bf16, tag="wo")
                bq_sb = consts.tile([P, nm], fp32, tag="bq")
                bk_sb = consts.tile([P, nm], fp32, tag="bk")
                bv_sb = consts.tile([P, nh, d], fp32, tag="bv")
                ones_sb = consts.tile([P, 1], bf16, tag="ones")

                for w_sb, w_h in ((wk_sb, wkT), (wq_sb, wqT)):
                    nc.scalar.dma_start(
                        out=w_sb[:, :, 0:P],
                        in_=w_h[:, 0:P].rearrange("(nb p) co -> p nb co", p=P))
                nc.sync.dma_start(out=bq_sb, in_=bqv[:, :])
                nc.sync.dma_start(out=bk_sb, in_=bkv[:, :])
                nc.vector.memset(ones_sb, 1.0)

                def emit_late_consts():
                    for w_sb, w_h in ((wk_sb, wkT), (wq_sb, wqT)):
                        if nm > 1:
                            nc.gpsimd.dma_start(
                                out=w_sb[:, :, P:cout],
                                in_=w_h[:, P:cout].rearrange(
                                    "(nb p) co -> p nb co", p=P))
                    nc.gpsimd.dma_start(
                        out=wv_sb,
                        in_=wvT[:, :].rearrange("(nb p) co -> p nb co", p=P))
                    nc.gpsimd.dma_start(
                        out=wo_sb,
                        in_=woT[:, :].rearrange("(nb p) co -> p nb co", p=P))
                    nc.gpsimd.dma_start(
                        out=bv_sb,
                        in_=bvv[:, :].rearrange("o (h e) -> o h e", h=nh)
                        .to_broadcast([P, nh, d]))

                # ---- persistent activations --------------------------------
                qh_sb = persist.tile([P, nm, tok], bf16, tag="qh")
                kh_sb = persist.tile([P, nm, tok], bf16, tag="kh")
                vh_all = persist.tile([P, nt, nh, d], bf16, tag="vh")
                att_sb = persist.tile([P, ntqb, ntb, 2 * P], bf16, tag="attn")
                att_T = persist.tile([P, nko, tok], bf16, tag="attT")

                def emit_qk_chunk(x_h, w_sb, b_sb, xh_sb, it, m, xtag,
                                  use_s=False):
                    xt = xstream.tile([P, ncin, sck], bf16, tag=xtag,
                                      name=f"xt_{xtag}_{it}_{m}")
                    nc.sync.dma_start(
                        out=xt,
                        in_=x_h[:, :].rearrange("(nb p) t -> p nb t", p=P)
                        [:, :, it * sck:(it + 1) * sck])
                    if use_s:
                        # pre-phase: borrow the scores banks (idle until the
                        # first pair) so "pp" can stay single-buffered.
                        ps = psum.tile([P, tqb], fp32, tag="s", bufs=2,
                                       name="psqk")
                        for half in range(sck // 512):
                            for ci in range(ncin):
                                nc.tensor.matmul(
                                    ps[:, half * 512:(half + 1) * 512],
                                    w_sb[:, ci, m * P:(m + 1) * P],
                                    xt[:, ci, half * 512:(half + 1) * 512],
                                    start=(ci == 0), stop=(ci == ncin - 1))
                        nc.vector.tensor_scalar(
                            out=xh_sb[:, m, it * sck:it * sck + sck],
                            in0=ps[:, 0:sck],
                            scalar1=b_sb[:, m:m + 1], scalar2=None,
                            op0=mybir.AluOpType.add)
                        return
                    for half in range(sck // 256):
                        ps = psum.tile([P, 256], fp32, tag="pp", bufs=1,
                                       name="psqk")
                        for ci in range(ncin):
                            nc.tensor.matmul(
                                ps, w_sb[:, ci, m * P:(m + 1) * P],
                                xt[:, ci, half * 256:(half + 1) * 256],
                                start=(ci == 0), stop=(ci == ncin - 1))
                        c0 = it * sck + half * 256
                        nc.vector.tensor_scalar(
                            out=xh_sb[:, m, c0:c0 + 256],
                            in0=ps, scalar1=b_sb[:, m:m + 1], scalar2=None,
                            op0=mybir.AluOpType.add)

                def emit_v_tile(it):
                    vt = vstream.tile([P, ncin, P], bf16, tag="vt",
                                      name=f"vt_{it}")
                    nc.gpsimd.dma_start(
                        out=vt,
                        in_=vT[:, :].rearrange("(nb p) t -> p nb t", p=P)
                        [:, :, it * P:(it + 1) * P])
                    ps = psum.tile([P, cout], fp32, tag="pp", bufs=1,
                                   name="psv")
                    for ci in range(ncin):
                        nc.tensor.matmul(ps, vt[:, ci, :], wv_sb[:, ci, :],
                                         start=(ci == 0),
                                         stop=(ci == ncin - 1))
                    nc.vector.tensor_tensor(
                        out=vh_all[:, it, :, :],
                        in0=ps.rearrange("p (h e) -> p h e", h=nh),
                        in1=bv_sb,
                        op=mybir.AluOpType.add)

                # m=0 K/Q chunks upfront (kh m0 gates the first scores).
                for it in range(tok // sck):
                    emit_qk_chunk(kT, wk_sb, bk_sb, kh_sb, it, 0, "xk",
                                  use_s=True)
                    emit_qk_chunk(qT, wq_sb, bq_sb, qh_sb, it, 0, "xq",
                                  use_s=True)
                emit_late_consts()

                # ---- attention ---------------------------------------------
                # pair order: (tb,hp) -- m=1 projections stream during
                # pair (1,0); V tiles during pair (0,0).
                if ntqb > 1:
                    order = [(0, 0), (1, 0), (0, 1), (1, 1)]
                else:
                    order = [(0, 0), (0, 1)] if nh > 2 else [(0, 0)]
                m1_chunks = []
                if nm > 1:
                    m1_chunks = [(kT, wk_sb, bk_sb, kh_sb, it, 1, "xk")
                                 for it in range(tok // sck)]
                    m1_chunks += [(qT, wq_sb, bq_sb, qh_sb, it, 1, "xq")
                                  for it in range(tok // sck)]

                def copy_to(ceng, out, in_):
                    if ceng is nc.scalar:
                        nc.scalar.activation(
                            out=out, in_=in_,
                            func=mybir.ActivationFunctionType.Copy)
                    else:
                        ceng.tensor_copy(out=out, in_=in_)

                deferred = []

                def pop_deferred(k):
                    for _ in range(k):
                        if deferred:
                            deferred.pop(0)()

                exp_bufs = 16

                for pi, (tb, hp) in enumerate(order):
                    m = hp if nm > 1 else 0
                    heads = ((2 * hp, 0), (2 * hp + 1, d))
                    exs = {}
                    last = (pi == len(order) - 1)
                    pv = psum.tile([P, 2, ntb, d], fp32, tag="pv", bufs=1,
                                   name=f"pv_{tb}_{hp}")
                    den = psum.tile([P, 2 * ntb], fp32, tag="den", bufs=1,
                                    name=f"den_{tb}_{hp}")

                    def pv_slot(ts, pv=pv, den=den, heads=heads, exs=exs):
                        # one long-lived accumulation group per PSUM bank:
                        # bank hh holds 8 [128,64] regions (start pending-
                        # zeroes the whole 2KB bank, so only the bank's first
                        # matmul starts, only its last stops).
                        for hh, (h, p0) in enumerate(heads):
                            e = exs[(h, ts)]
                            for t in range(ntb):
                                nc.tensor.matmul(
                                    pv[:, hh, t, :], e[:, t * P:(t + 1) * P],
                                    vh_all[:, ts, h, :],
                                    start=(ts == 0 and t == 0),
                                    stop=(ts == nt - 1 and t == ntb - 1),
                                    skip_group_check=not (
                                        (ts == 0 and t == 0)
                                        or (ts == nt - 1 and t == ntb - 1)))
                                col = hh * ntb + t
                                nc.tensor.matmul(
                                    den[:, col:col + 1],
                                    e[:, t * P:(t + 1) * P], ones_sb,
                                    start=(ts == 0 and hh == 0 and t == 0),
                                    stop=(ts == nt - 1 and hh == 1
                                          and t == ntb - 1),
                                    skip_group_check=not (
                                        (ts == 0 and hh == 0 and t == 0)
                                        or (ts == nt - 1 and hh == 1
                                            and t == ntb - 1)))

                    pv_next = 0
                    for i in range(nt):
                        for hh, (h, p0) in enumerate(heads):
                            s_ps = psum.tile([P, tqb], fp32, tag="s",
                                             bufs=2, name="s_ps")
                            for cc in range(csk):
                                q0 = tb * tqb + cc * sck
                                nc.tensor.matmul(
                                    s_ps[:, cc * sck:(cc + 1) * sck],
                                    kh_sb[p0:p0 + d, m, i * P:(i + 1) * P],
                                    qh_sb[p0:p0 + d, m, q0:q0 + sck],
                                    start=True, stop=True)
                            ex = expool.tile([P, tqb], bf16, tag="ex",
                                             bufs=exp_bufs,
                                             name=f"ex_{h}_{i}")
                            nc.scalar.activation(
                                out=ex, in_=s_ps,
                                func=mybir.ActivationFunctionType.Exp,
                                scale=float(d) ** -0.5)
                            exs[(h, i)] = ex
                        if pi == 0:
                            emit_v_tile(i)
                        if pi == 1 and m1_chunks and i % 2 == 0:
                            emit_qk_chunk(*m1_chunks.pop(0))
                        budget = 2
                        while pv_next <= i - 1 and budget:
                            pv_slot(pv_next)
                            pv_next += 1
                            budget -= 1
                        pop_deferred(2 if last else 3)
                    while pv_next < nt:
                        pv_slot(pv_next)
                        pv_next += 1

                    # ---- drain: normalize (+ transpose/outproj when hp==1)
                    drain = []

                    def mk_norm(tb, hp, hh, h, t, pv=pv, den=den):
                        def go(ceng=nc.vector):
                            col = hh * ntb + t
                            rec = smalls.tile([P, 1], fp32, tag="rec",
                                              name=f"rec_{tb}_{h}_{t}")
                            nc.vector.reciprocal_approx_fast(
                                out=rec, in_=den[:, col:col + 1])
                            nc.vector.tensor_scalar(
                                out=att_sb[:, tb, t,
                                           (2 * hp + hh) * d:
                                           (2 * hp + hh + 1) * d],
                                in0=pv[:, hh, t, :], scalar1=rec,
                                scalar2=None, op0=mybir.AluOpType.mult)
                        return go

                    def mk_transpose(tb, t):
                        def go(ceng=nc.vector):
                            tbase = tb * tqb + t * P
                            for ko in range(nko):
                                nc.sync.dma_start_transpose(
                                    out=att_T[:, ko, tbase:tbase + P],
                                    in_=att_sb[:, tb, t,
                                               ko * P:(ko + 1) * P])
                        return go

                    def mk_outproj(tb, t, tail=False):
                        def go(ceng=nc.vector):
                            tbase = tb * tqb + t * P
                            o_sb = ostage.tile([P, cin], bf16, tag="ost",
                                               name=f"ost_{tb}_{t}")
                            if tail:
                                # scores are done -- reuse their banks for
                                # [P,1024] chunks and one big evacuation copy.
                                for n2 in range(cin // tqb):
                                    ps = psum.tile([P, tqb], fp32, tag="s",
                                                   bufs=2, name="pso")
                                    for half in range(tqb // 512):
                                        for ko in range(nko):
                                            nc.tensor.matmul(
                                                ps[:, half * 512:
                                                   (half + 1) * 512],
                                                att_T[:, ko,
                                                      tbase:tbase + P],
                                                wo_sb[:, ko,
                                                      n2 * tqb + half * 512:
                                                      n2 * tqb
                                                      + (half + 1) * 512],
                                                start=(ko == 0),
                                                stop=(ko == nko - 1))
                                    copy_to(ceng,
                                            o_sb[:, n2 * tqb:(n2 + 1) * tqb],
                                            ps)
                            else:
                                for n in range(nob):
                                    ps = psum.tile([P, 256], fp32, tag="pp",
                                                   bufs=1, name="pso")
                                    for ko in range(nko):
                                        nc.tensor.matmul(
                                            ps,
                                            att_T[:, ko, tbase:tbase + P],
                                            wo_sb[:, ko,
                                                  n * 256:(n + 1) * 256],
                                            start=(ko == 0),
                                            stop=(ko == nko - 1))
                                    copy_to(ceng,
                                            o_sb[:, n * 256:(n + 1) * 256],
                                            ps)
                            nc.sync.dma_start(
                                out=outp[tbase:tbase + P, :], in_=o_sb)
                        return go

                    for t in range(ntb):
                        for hh, (h, p0) in enumerate(heads):
                            drain.append(mk_norm(tb, hp, hh, h, t))
                        if hp == nh // 2 - 1:
                            drain.append(mk_transpose(tb, t))
                            drain.append(mk_outproj(tb, t, tail=last))

                    if not last:
                        deferred.extend(drain)
                    else:
                        # tail: alternate evacuation copies between DVE and
                        # the now-idle ScalarE.
                        pop_deferred(len(deferred))
                        for j, work in enumerate(drain):
                            work(ceng=(nc.scalar if j % 2 else nc.vector))

    nc.compile()
    return nc


def _host_inputs(q, k, v, Wq, Wk, Wv, Wo, bq, bk, bv,
                 tok=TOKENS, cin=C, cout=COUT, ngroup=NGROUP, ncores=NCORES):
    """Build per-core in_maps (host-side shard + transpose + bf16 cast)."""
    nm = max(1, cout // P)
    xT = {}
    for b in range(q.shape[0]):
        xT[('q', b)] = np.ascontiguousarray(q[b].T).astype(BF16)
        xT[('k', b)] = np.ascontiguousarray(k[b].T).astype(BF16)
        xT[('v', b)] = np.ascontiguousarray(v[b].T).astype(BF16)
    in_maps = []
    for core in range(ncores):
        b, g = core // ngroup, core % ngroup
        sl = slice(g * cout, (g + 1) * cout)
        in_maps.append({
            "qT": xT[('q', b)],
            "kT": xT[('k', b)],
            "vT": xT[('v', b)],
            "wqT": np.ascontiguousarray(Wq[sl, :].T).astype(BF16),
            "wkT": np.ascontiguousarray(Wk[sl, :].T).astype(BF16),
            "wvT": np.ascontiguousarray(Wv[sl, :].T).astype(BF16),
            "woT": np.ascontiguousarray(Wo[:, sl].T).astype(BF16),
            "bqv": np.ascontiguousarray(
                bq[sl].reshape(nm, P).T).astype(np.float32),
            "bkv": np.ascontiguousarray(
                bk[sl].reshape(nm, P).T).astype(np.float32),
            "bvv": np.ascontiguousarray(bv[sl][None, :]).astype(np.float32),
        })
    return in_maps


_NC_CACHE = {}


def _get_nc():
    if "nc" not in _NC_CACHE:
        _NC_CACHE["nc"] = build_nc()
    return _NC_CACHE["nc"]


def kernel(q, k, v, Wq, bq, Wk, bk, Wv, bv, Wo, bo):
    from concourse.bass_utils import run_bass_kernel_spmd

    q = np.asarray(q, dtype=np.float32)
    k = np.asarray(k, dtype=np.float32)
    v = np.asarray(v, dtype=np.float32)
    nc = _get_nc()
    in_maps = _host_inputs(q, k, v,
                           np.asarray(Wq, np.float32), np.asarray(Wk, np.float32),
                           np.asarray(Wv, np.float32), np.asarray(Wo, np.float32),
                           np.asarray(bq, np.float32), np.asarray(bk, np.float32),
                           np.asarray(bv, np.float32))
    res = run_bass_kernel_spmd(nc, in_maps, core_ids=list(range(NCORES)))
    parts = [np.asarray(r["outp"], dtype=np.float32) for r in res.results]
    out = np.stack(
        [sum(parts[b * NGROUP:(b + 1) * NGROUP]) for b in range(B)], axis=0)
    out = out + np.asarray(bo, np.float32)[None, None, :]
    return out.astype(np.float32)



# revision 18
# speedup vs baseline: 1.5287x; 1.5287x over previous
"""CrossAttention TRN2 kernel (v2: transposed-PV with self-interleaved
accumulation).

Full-input contract: kernel(**inputs) takes the unsharded numpy inputs of
  reference.py (q,k,v [2,2048,1024] fp32; Wq/Wk/Wv/Wo [1024,1024]; biases)
and returns the full [2,2048,1024] fp32 output.

Sharding: 8 cores = 2 batch groups x 4 head groups (tensor parallel over
heads).  Core c handles batch c//4 and heads [4*(c%4), 4*(c%4)+4).
Each core computes its heads' Q/K/V projections, attention, and a partial
output projection (row-slice of Wo); the host sums the 4 partials per batch
(no on-device collectives needed).

Per-core dataflow (all matmuls bf16 with fp32 PSUM accumulation):
  - host pre-transposes/casts activations (q^T,k^T,v^T [cin, tok] bf16) and
    weight slices, so contraction dims land on SBUF partitions directly.
  - scores are computed transposed ([ts, tq]): stationary k-tile [64,128],
    moving q [64,512].  exp on ScalarE (scale 1/8 folded), FD=1024.
  - PV runs TRANSPOSED: out^T[tq,d] accumulates over ts tiles with the exp
    tile as stationary [128,128] and v[ts,d] as moving [128,64] -- half the
    PE rows of the [65,512]-oriented PV.  A parallel 1-wide matmul against a
    ones vector accumulates the softmax denominator per tq ON PARTITIONS,
    so normalization is a per-partition reciprocal + tensor_scalar (no DMA
    broadcast round-trip).  PV consumes each exp tile the step after it is
    produced (self-interleaved), so no bulk PV drain remains at the end.
  - normalized att tiles [tq,256] are PE-transposed (identity matmul) back
    to [cout,tq] for the output projection, which is unchanged.
  - pair order (tb,hp) = (0,0),(1,0),(0,1),(1,1); drains are deferred into
    the next pair's phase-1 steps; the last pair's drain splits its PSUM
    evacuation copies between DVE and the (then idle) ScalarE.
  - q/k/v bias adds run on Pool (gpsimd) to keep DVE under the ScalarE
    roofline; ScalarE exp (~133 us) is the modeled bottleneck.
"""

import os
import numpy as np
import ml_dtypes

BF16 = ml_dtypes.bfloat16

B, TOKENS, C = 2, 2048, 1024
NHEAD, D = 16, 64
NCORES = 8
NGROUP = 4                # head groups (cores per batch)
COUT = C // NGROUP        # 256 head-channels per core
NH = NHEAD // NGROUP      # 4 heads per core

P = 128                   # SBUF partitions

# scores arrive in PSUM pre-scaled by 1/EXP_SCALE (folded into Wq/bq on the
# host): y = z/128 with z the scaled score.  ScalarE computes exp(y*128);
# DVE computes (1+y)^128 via 7 squarings (rel err ~ z^2/256, |z|<2.8 here).
EXP_SCALE = 1024.0        # host folds 1/1024 into Wq, bq
_DVE_EXP_NAME = "TENSOR_MASK"   # registry slot reused for the exp op


def _install_dve_exp():
    """Register the exp-by-squaring op in the custom-DVE registry (in-place
    swap of an existing slot; per-NEFF uop tables are generated from it)."""
    import numpy as np
    import concourse.dve_ops as dvo
    from concourse.dve_spec import Spec, Src0, C0, sq, lower as dve_lower
    from concourse.dve_uop import DveOpSpec

    cur = next(o for o in dvo.OPS if o.name == _DVE_EXP_NAME)
    if getattr(cur, "_is_exp128", False):
        return cur

    def ref(in0, in1, s0, s1, imm2):
        r = in0.astype(np.float32) + s0
        for _ in range(7):
            r = r * r
        return r

    body = Src0 + C0
    for _ in range(7):
        body = sq(body)
    spec = Spec(body=body, reference=ref)
    shas = {}
    for ver in ("v3", "v4"):
        uops = dve_lower(spec, ver=ver)
        shas[ver] = DveOpSpec(name=_DVE_EXP_NAME,
                              opcode=dvo.get_dve_sub_opcode(_DVE_EXP_NAME),
                              uops=uops, rd1_en=False).sha(ver)
    op = dvo.DveOp(name=_DVE_EXP_NAME, spec=spec, subdim=False,
                   uops_sha=shas)
    object.__setattr__(op, "_is_exp128", True)
    dvo._COMPILE_CACHE.pop((_DVE_EXP_NAME, "v3"), None)
    dvo._COMPILE_CACHE.pop((_DVE_EXP_NAME, "v4"), None)
    idx = next(i for i, o in enumerate(dvo.OPS) if o.name == _DVE_EXP_NAME)
    dvo.OPS[idx] = op
    return op


# exp tiles with (2*i+hh) % 16 in DVE_SLOTS run on DVE; rest on ScalarE.
DVE_SLOTS = (1, 4, 7, 10, 12, 15)


def build_nc(tok=TOKENS, cin=C, cout=COUT, nh=NH, reps=1, diag=None,
             dve_slots=DVE_SLOTS, bias_pool=False, evac_pool=False,
             s128=True, qk_bias_scalar=True, evac_alt=True):
    """Emit the per-core Bass module. d=64 fixed; cout = nh*64.
    reps>1 replicates the whole body (timing builds only): per-rep marginal
    time = steady-state kernel time with per-exec dispatch overhead removed.
    diag: timing-only work-gutting variants (output garbage):
      'noexp'    - exp activations shrunk to [128,32] (ScalarE ~0)
      'halfscore'- only half of each scores tile computed (PE -27us)
      'halfpv'   - pv_slot processes one head only (PE -14us)
      'halfproj' - m=1 q/k projection chunks skipped (PE -14us)"""
    import concourse.bacc as bacc
    import concourse.tile as tile
    import concourse.mybir as mybir
    from concourse import masks

    d = D
    assert cout == nh * d
    ncin = cin // P               # cin tiles (contraction)
    nt = tok // P                 # ts tiles
    nm = max(1, cout // P)        # 128-wide cout chunks (qhT/khT)
    tqb = min(1024, tok)          # tq block (exp FD)
    ntqb = tok // tqb
    ntb = tqb // P                # tq tiles per block (8)
    sck = min(512, tok)           # scores moving chunk
    csk = tqb // sck
    nko = max(1, cout // P)       # out-proj contraction tiles
    nob = max(1, cin // 256)      # out-proj 256-wide chunks

    fp32 = mybir.dt.float32
    bf16 = mybir.dt.bfloat16

    dve_exp_op = _install_dve_exp() if dve_slots else None

    nc = bacc.Bacc("TRN2", target_bir_lowering=False, debug=False)

    qT = nc.dram_tensor("qT", [cin, tok], bf16, kind="ExternalInput")
    kT = nc.dram_tensor("kT", [cin, tok], bf16, kind="ExternalInput")
    vT = nc.dram_tensor("vT", [cin, tok], bf16, kind="ExternalInput")
    wqT = nc.dram_tensor("wqT", [cin, cout], bf16, kind="ExternalInput")
    wkT = nc.dram_tensor("wkT", [cin, cout], bf16, kind="ExternalInput")
    wvT = nc.dram_tensor("wvT", [cin, cout], bf16, kind="ExternalInput")
    woT = nc.dram_tensor("woT", [cout, cin], bf16, kind="ExternalInput")
    bqv = nc.dram_tensor("bqv", [P, nm], fp32, kind="ExternalInput")
    bkv = nc.dram_tensor("bkv", [P, nm], fp32, kind="ExternalInput")
    bvv = nc.dram_tensor("bvv", [1, cout], fp32, kind="ExternalInput")
    outp = nc.dram_tensor("outp", [tok, cin], bf16, kind="ExternalOutput")

    with tile.TileContext(nc) as tc:
        from contextlib import ExitStack
        with ExitStack() as ctx:
            consts = ctx.enter_context(tc.tile_pool(name="consts", bufs=1))
            xstream = ctx.enter_context(tc.tile_pool(name="xstream", bufs=2))
            vstream = ctx.enter_context(tc.tile_pool(name="vstream", bufs=2))
            persist = ctx.enter_context(tc.tile_pool(name="persist", bufs=1))
            expool = ctx.enter_context(tc.tile_pool(name="expool", bufs=4))
            smalls = ctx.enter_context(tc.tile_pool(name="smalls", bufs=8))
            ostage = ctx.enter_context(tc.tile_pool(name="ostage", bufs=3))
            psum = ctx.enter_context(
                tc.tile_pool(name="psum", bufs=1, space="PSUM"))

            for _rep in range(reps):
                # ---- constants (K/Q m=0 weights first — gate the first exp)
                wq_sb = consts.tile([P, ncin, cout], bf16, tag="wq")
                wk_sb = consts.tile([P, ncin, cout], bf16, tag="wk")
                wv_sb = consts.tile([P, ncin, cout], bf16, tag="wv")
                wo_sb = consts.tile([P, nko, cin], bf16, tag="wo")
                bq_sb = consts.tile([P, nm], fp32, tag="bq")
                bk_sb = consts.tile([P, nm], fp32, tag="bk")
                bv_sb = consts.tile([P, nh, d], fp32, tag="bv")
                ones_sb = consts.tile([P, 1], bf16, tag="ones")

                for w_sb, w_h in ((wk_sb, wkT), (wq_sb, wqT)):
                    nc.scalar.dma_start(
                        out=w_sb[:, :, 0:P],
                        in_=w_h[:, 0:P].rearrange("(nb p) co -> p nb co", p=P))
                nc.sync.dma_start(out=bq_sb, in_=bqv[:, :])
                nc.sync.dma_start(out=bk_sb, in_=bkv[:, :])
                nc.vector.memset(ones_sb, 1.0)

                def emit_late_consts():
                    for w_sb, w_h in ((wk_sb, wkT), (wq_sb, wqT)):
                        if nm > 1:
                            nc.gpsimd.dma_start(
                                out=w_sb[:, :, P:cout],
                                in_=w_h[:, P:cout].rearrange(
                                    "(nb p) co -> p nb co", p=P))
                    nc.gpsimd.dma_start(
                        out=wv_sb,
                        in_=wvT[:, :].rearrange("(nb p) co -> p nb co", p=P))
                    nc.gpsimd.dma_start(
                        out=wo_sb,
                        in_=woT[:, :].rearrange("(nb p) co -> p nb co", p=P))
                    nc.gpsimd.dma_start(
                        out=bv_sb,
                        in_=bvv[:, :].rearrange("o (h e) -> o h e", h=nh)
                        .to_broadcast([P, nh, d]))

                # ---- persistent activations --------------------------------
                qh_sb = persist.tile([P, nm, tok], bf16, tag="qh")
                # kh: with s128, per-head 128-row slots (data rows h%2*64,
                # the other 64 rows zeroed) so scores contract over 128.
                if s128:
                    kh_sb = persist.tile([P, nh, tok], bf16, tag="kh")
                    for h in range(nh):
                        z0 = (1 - h % 2) * 64
                        nc.gpsimd.memset(kh_sb[z0:z0 + 64, h, :], 0.0)
                else:
                    kh_sb = persist.tile([P, nm, tok], bf16, tag="kh")
                vh_all = persist.tile([P, nt, nh, d], bf16, tag="vh")
                att_sb = persist.tile([P, ntqb, ntb, 2 * P], bf16, tag="attn")
                att_T = persist.tile([P, nko, tok], bf16, tag="attT")

                def bias_add(out, in0, b_ap):
                    if qk_bias_scalar:
                        nc.scalar.add(out=out, in_=in0, add=b_ap)
                    else:
                        nc.vector.tensor_scalar(
                            out=out, in0=in0, scalar1=b_ap, scalar2=None,
                            op0=mybir.AluOpType.add)

                def qk_evac(is_k, xh_sb, ps_slice, m, c0, cn, b_sb):
                    if is_k and s128:
                        for hh in range(2):
                            p0 = hh * 64
                            bias_add(
                                out=xh_sb[p0:p0 + 64, 2 * m + hh,
                                          c0:c0 + cn],
                                in0=ps_slice[p0:p0 + 64, :],
                                b_ap=b_sb[p0:p0 + 64, m:m + 1])
                    else:
                        bias_add(out=xh_sb[:, m, c0:c0 + cn],
                                 in0=ps_slice, b_ap=b_sb[:, m:m + 1])

                def emit_qk_chunk(x_h, w_sb, b_sb, xh_sb, it, m, xtag,
                                  use_s=False):
                    is_k = xtag == "xk"
                    xt = xstream.tile([P, ncin, sck], bf16, tag=xtag,
                                      name=f"xt_{xtag}_{it}_{m}")
                    nc.sync.dma_start(
                        out=xt,
                        in_=x_h[:, :].rearrange("(nb p) t -> p nb t", p=P)
                        [:, :, it * sck:(it + 1) * sck])
                    if use_s:
                        # pre-phase: borrow the scores banks (idle until the
                        # first pair) so "pp" can stay single-buffered.
                        ps = psum.tile([P, tqb], fp32, tag="s", bufs=2,
                                       name="psqk")
                        for half in range(sck // 512):
                            for ci in range(ncin):
                                nc.tensor.matmul(
                                    ps[:, half * 512:(half + 1) * 512],
                                    w_sb[:, ci, m * P:(m + 1) * P],
                                    xt[:, ci, half * 512:(half + 1) * 512],
                                    start=(ci == 0), stop=(ci == ncin - 1))
                        qk_evac(is_k, xh_sb, ps[:, 0:sck], m,
                                it * sck, sck, b_sb)
                        return
                    for half in range(sck // 256):
                        ps = psum.tile([P, 256], fp32, tag="pp", bufs=1,
                                       name="psqk")
                        for ci in range(ncin):
                            nc.tensor.matmul(
                                ps, w_sb[:, ci, m * P:(m + 1) * P],
                                xt[:, ci, half * 256:(half + 1) * 256],
                                start=(ci == 0), stop=(ci == ncin - 1))
                        c0 = it * sck + half * 256
                        qk_evac(is_k, xh_sb, ps, m, c0, 256, b_sb)

                def emit_v_tile(it):
                    vt = vstream.tile([P, ncin, P], bf16, tag="vt",
                                      name=f"vt_{it}")
                    nc.gpsimd.dma_start(
                        out=vt,
                        in_=vT[:, :].rearrange("(nb p) t -> p nb t", p=P)
                        [:, :, it * P:(it + 1) * P])
                    ps = psum.tile([P, cout], fp32, tag="pp", bufs=1,
                                   name="psv")
                    for ci in range(ncin):
                        nc.tensor.matmul(ps, vt[:, ci, :], wv_sb[:, ci, :],
                                         start=(ci == 0),
                                         stop=(ci == ncin - 1))
                    (nc.gpsimd if bias_pool else nc.vector).tensor_tensor(
                        out=vh_all[:, it, :, :],
                        in0=ps.rearrange("p (h e) -> p h e", h=nh),
                        in1=bv_sb,
                        op=mybir.AluOpType.add)

                # m=0 K/Q chunks upfront (kh m0 gates the first scores).
                for it in range(tok // sck):
                    emit_qk_chunk(kT, wk_sb, bk_sb, kh_sb, it, 0, "xk",
                                  use_s=True)
                    emit_qk_chunk(qT, wq_sb, bq_sb, qh_sb, it, 0, "xq",
                                  use_s=True)
                emit_late_consts()

                # ---- attention ---------------------------------------------
                # pair order: (tb,hp) -- m=1 projections stream during
                # pair (1,0); V tiles during pair (0,0).
                if ntqb > 1:
                    order = [(0, 0), (1, 0), (0, 1), (1, 1)]
                else:
                    order = [(0, 0), (0, 1)] if nh > 2 else [(0, 0)]
                m1_chunks = []
                if nm > 1 and diag != "halfproj":
                    m1_chunks = [(kT, wk_sb, bk_sb, kh_sb, it, 1, "xk")
                                 for it in range(tok // sck)]
                    m1_chunks += [(qT, wq_sb, bq_sb, qh_sb, it, 1, "xq")
                                  for it in range(tok // sck)]

                def copy_to(ceng, out, in_):
                    if ceng is nc.scalar:
                        nc.scalar.activation(
                            out=out, in_=in_,
                            func=mybir.ActivationFunctionType.Copy)
                    else:
                        ceng.tensor_copy(out=out, in_=in_)

                deferred = []

                def pop_deferred(k):
                    for _ in range(k):
                        if deferred:
                            deferred.pop(0)()

                exp_bufs = 16

                for pi, (tb, hp) in enumerate(order):
                    m = hp if nm > 1 else 0
                    heads = ((2 * hp, 0), (2 * hp + 1, d))
                    exs = {}
                    last = (pi == len(order) - 1)
                    pv = psum.tile([P, 2, ntb, d], fp32, tag="pv", bufs=1,
                                   name=f"pv_{tb}_{hp}")
                    den = psum.tile([P, 2 * ntb], fp32, tag="den", bufs=1,
                                    name=f"den_{tb}_{hp}")

                    def pv_slot(ts, pv=pv, den=den, heads=heads, exs=exs):
                        # one long-lived accumulation group per PSUM bank:
                        # bank hh holds 8 [128,64] regions (start pending-
                        # zeroes the whole 2KB bank, so only the bank's first
                        # matmul starts, only its last stops).
                        den_last_hh = 0 if diag == "halfpv" else 1
                        for hh, (h, p0) in enumerate(heads):
                            e = exs[(h, ts)]
                            for t in range(ntb):
                                if not (diag == "halfpv" and hh == 1):
                                    nc.tensor.matmul(
                                        pv[:, hh, t, :],
                                        e[:, t * P:(t + 1) * P],
                                        vh_all[:, ts, h, :],
                                        start=(ts == 0 and t == 0),
                                        stop=(ts == nt - 1 and t == ntb - 1),
                                        skip_group_check=not (
                                            (ts == 0 and t == 0)
                                            or (ts == nt - 1
                                                and t == ntb - 1)))
                                col = hh * ntb + t
                                nc.tensor.matmul(
                                    den[:, col:col + 1],
                                    e[:, t * P:(t + 1) * P], ones_sb,
                                    start=(ts == 0 and hh == 0 and t == 0),
                                    stop=(ts == nt - 1 and hh == den_last_hh
                                          and t == ntb - 1),
                                    skip_group_check=not (
                                        (ts == 0 and hh == 0 and t == 0)
                                        or (ts == nt - 1 and hh == den_last_hh
                                            and t == ntb - 1)))

                    pv_next = 0
                    for i in range(nt):
                        for hh, (h, p0) in enumerate(heads):
                            s_ps = psum.tile([P, tqb], fp32, tag="s",
                                             bufs=2, name="s_ps")
                            ncc = 1 if diag == "halfscore" else csk
                            for cc in range(ncc):
                                q0 = tb * tqb + cc * sck
                                if s128:
                                    # 128-row contraction: kh slot h has the
                                    # other head's rows zeroed, so the packed
                                    # qh (both heads) contracts correctly.
                                    nc.tensor.matmul(
                                        s_ps[:, cc * sck:(cc + 1) * sck],
                                        kh_sb[:, h, i * P:(i + 1) * P],
                                        qh_sb[:, m, q0:q0 + sck],
                                        start=True, stop=True)
                                else:
                                    nc.tensor.matmul(
                                        s_ps[:, cc * sck:(cc + 1) * sck],
                                        kh_sb[p0:p0 + d, m,
                                              i * P:(i + 1) * P],
                                        qh_sb[p0:p0 + d, m, q0:q0 + sck],
                                        start=True, stop=True)
                            ex = expool.tile([P, tqb], bf16, tag="ex",
                                             bufs=exp_bufs,
                                             name=f"ex_{h}_{i}")
                            sc_scale = EXP_SCALE * float(d) ** -0.5
                            if diag == "noexp":
                                nc.scalar.activation(
                                    out=ex[:, 0:32], in_=s_ps[:, 0:32],
                                    func=mybir.ActivationFunctionType.Exp,
                                    scale=sc_scale)
                            elif (dve_slots
                                  and (2 * i + hh) % 16 in dve_slots):
                                nc.vector._custom_dve(
                                    dve_exp_op, out=ex, in0=s_ps, s0=1.0)
                            else:
                                nc.scalar.activation(
                                    out=ex, in_=s_ps,
                                    func=mybir.ActivationFunctionType.Exp,
                                    scale=sc_scale)
                            exs[(h, i)] = ex
                        if pi == 0:
                            emit_v_tile(i)
                        if pi == 1 and m1_chunks and i % 2 == 0:
                            emit_qk_chunk(*m1_chunks.pop(0))
                        budget = 2
                        while pv_next <= i - 1 and budget:
                            pv_slot(pv_next)
                            pv_next += 1
                            budget -= 1
                        pop_deferred(2 if last else 3)
                    while pv_next < nt:
                        pv_slot(pv_next)
                        pv_next += 1

                    # ---- drain: normalize (+ transpose/outproj when hp==1)
                    drain = []

                    def mk_norm(tb, hp, hh, h, t, pv=pv, den=den):
                        def go(ceng=nc.vector):
                            col = hh * ntb + t
                            rec = smalls.tile([P, 1], fp32, tag="rec",
                                              name=f"rec_{tb}_{h}_{t}")
                            nc.vector.reciprocal_approx_fast(
                                out=rec, in_=den[:, col:col + 1])
                            att_dst = att_sb[:, tb, t,
                                             (2 * hp + hh) * d:
                                             (2 * hp + hh + 1) * d]
                            if evac_alt and (t + hh) % 2:
                                nc.scalar.mul(out=att_dst,
                                              in_=pv[:, hh, t, :], mul=rec)
                            else:
                                nc.vector.tensor_scalar(
                                    out=att_dst,
                                    in0=pv[:, hh, t, :], scalar1=rec,
                                    scalar2=None, op0=mybir.AluOpType.mult)
                        return go

                    def mk_transpose(tb, t):
                        def go(ceng=nc.vector):
                            tbase = tb * tqb + t * P
                            for ko in range(nko):
                                nc.sync.dma_start_transpose(
                                    out=att_T[:, ko, tbase:tbase + P],
                                    in_=att_sb[:, tb, t,
                                               ko * P:(ko + 1) * P])
                        return go

                    def mk_outproj(tb, t, tail=False):
                        def go(ceng=None):
                            if ceng is None:
                                ceng = (nc.scalar
                                        if (evac_alt and t % 2)
                                        else nc.vector)
                            tbase = tb * tqb + t * P
                            o_sb = ostage.tile([P, cin], bf16, tag="ost",
                                               name=f"ost_{tb}_{t}")
                            if tail:
                                # scores are done -- reuse their banks for
                                # [P,1024] chunks and one big evacuation copy.
                                for n2 in range(cin // tqb):
                                    ps = psum.tile([P, tqb], fp32, tag="s",
                                                   bufs=2, name="pso")
                                    for half in range(tqb // 512):
                                        for ko in range(nko):
                                            nc.tensor.matmul(
                                                ps[:, half * 512:
                                                   (half + 1) * 512],
                                                att_T[:, ko,
                                                      tbase:tbase + P],
                                                wo_sb[:, ko,
                                                      n2 * tqb + half * 512:
                                                      n2 * tqb
                                                      + (half + 1) * 512],
                                                start=(ko == 0),
                                                stop=(ko == nko - 1))
                                    copy_to(ceng,
                                            o_sb[:, n2 * tqb:(n2 + 1) * tqb],
                                            ps)
                            else:
                                for n in range(nob):
                                    ps = psum.tile([P, 256], fp32, tag="pp",
                                                   bufs=1, name="pso")
                                    for ko in range(nko):
                                        nc.tensor.matmul(
                                            ps,
                                            att_T[:, ko, tbase:tbase + P],
                                            wo_sb[:, ko,
                                                  n * 256:(n + 1) * 256],
                                            start=(ko == 0),
                                            stop=(ko == nko - 1))
                                    copy_to(ceng,
                                            o_sb[:, n * 256:(n + 1) * 256],
                                            ps)
                            nc.sync.dma_start(
                                out=outp[tbase:tbase + P, :], in_=o_sb)
                        return go

                    for t in range(ntb):
                        for hh, (h, p0) in enumerate(heads):
                            drain.append(mk_norm(tb, hp, hh, h, t))
                        if hp == nh // 2 - 1:
                            drain.append(mk_transpose(tb, t))
                            drain.append(mk_outproj(tb, t, tail=last))

                    if not last:
                        deferred.extend(drain)
                    else:
                        # tail: alternate evacuation copies between DVE and
                        # the now-idle ScalarE.
                        pop_deferred(len(deferred))
                        for j, work in enumerate(drain):
                            work(ceng=(nc.scalar if j % 2 else nc.vector))

    nc.compile()
    return nc


def _host_inputs(q, k, v, Wq, Wk, Wv, Wo, bq, bk, bv,
                 tok=TOKENS, cin=C, cout=COUT, ngroup=NGROUP, ncores=NCORES):
    """Build per-core in_maps (host-side shard + transpose + bf16 cast)."""
    nm = max(1, cout // P)
    xT = {}
    for b in range(q.shape[0]):
        xT[('q', b)] = np.ascontiguousarray(q[b].T).astype(BF16)
        xT[('k', b)] = np.ascontiguousarray(k[b].T).astype(BF16)
        xT[('v', b)] = np.ascontiguousarray(v[b].T).astype(BF16)
    in_maps = []
    for core in range(ncores):
        b, g = core // ngroup, core % ngroup
        sl = slice(g * cout, (g + 1) * cout)
        in_maps.append({
            "qT": xT[('q', b)],
            "kT": xT[('k', b)],
            "vT": xT[('v', b)],
            "wqT": np.ascontiguousarray(
                Wq[sl, :].T / EXP_SCALE).astype(BF16),
            "wkT": np.ascontiguousarray(Wk[sl, :].T).astype(BF16),
            "wvT": np.ascontiguousarray(Wv[sl, :].T).astype(BF16),
            "woT": np.ascontiguousarray(Wo[:, sl].T).astype(BF16),
            "bqv": np.ascontiguousarray(
                bq[sl].reshape(nm, P).T / EXP_SCALE).astype(np.float32),
            "bkv": np.ascontiguousarray(
                bk[sl].reshape(nm, P).T).astype(np.float32),
            "bvv": np.ascontiguousarray(bv[sl][None, :]).astype(np.float32),
        })
    return in_maps


_NC_CACHE = {}


def _get_nc():
    if "nc" not in _NC_CACHE:
        _NC_CACHE["nc"] = build_nc()
    return _NC_CACHE["nc"]


def kernel(q, k, v, Wq, bq, Wk, bk, Wv, bv, Wo, bo):
    from concourse.bass_utils import run_bass_kernel_spmd

    q = np.asarray(q, dtype=np.float32)
    k = np.asarray(k, dtype=np.float32)
    v = np.asarray(v, dtype=np.float32)
    nc = _get_nc()
    in_maps = _host_inputs(q, k, v,
                           np.asarray(Wq, np.float32), np.asarray(Wk, np.float32),
                           np.asarray(Wv, np.float32), np.asarray(Wo, np.float32),
                           np.asarray(bq, np.float32), np.asarray(bk, np.float32),
                           np.asarray(bv, np.float32))
    res = run_bass_kernel_spmd(nc, in_maps, core_ids=list(range(NCORES)))
    parts = [np.asarray(r["outp"], dtype=np.float32) for r in res.results]
    out = np.stack(
        [sum(parts[b * NGROUP:(b + 1) * NGROUP]) for b in range(B)], axis=0)
    out = out + np.asarray(bo, np.float32)[None, None, :]
    return out.astype(np.float32)



# revision 34
# speedup vs baseline: 4.9772x; 3.2558x over previous
"""CrossAttention TRN2 kernel (v3: 128-contraction scores + coarse norm).

v3 changes over v2:
  - s128: kh is stored per-head in 128-row zero-padded slots so every
    scores matmul contracts over the full 128 partitions (measured ~2x
    faster than the 64-row contraction on HW); qh stays packed (the
    zeroed stationary rows make the extra qh rows inert).
  - scores arrive in PSUM pre-scaled by 1/1024 (folded into Wq/bq on the
    host); ScalarE exp uses scale=128.  A custom DVE exp-by-squaring op
    ((1+y)^128, 8-stage pipeline) is available behind dve_slots= for
    offloading exp, but measured slower than ScalarE-only on HW, so
    defaults keep all exp on ScalarE.
  - coarse_norm: softmax normalization as one [P,ntb] reciprocal + one
    broadcast tensor_tensor per (pair, head) instead of per-tile ops.


Full-input contract: kernel(**inputs) takes the unsharded numpy inputs of
  reference.py (q,k,v [2,2048,1024] fp32; Wq/Wk/Wv/Wo [1024,1024]; biases)
and returns the full [2,2048,1024] fp32 output.

Sharding: 8 cores = 2 batch groups x 4 head groups (tensor parallel over
heads).  Core c handles batch c//4 and heads [4*(c%4), 4*(c%4)+4).
Each core computes its heads' Q/K/V projections, attention, and a partial
output projection (row-slice of Wo); the host sums the 4 partials per batch
(no on-device collectives needed).

Per-core dataflow (all matmuls bf16 with fp32 PSUM accumulation):
  - host pre-transposes/casts activations (q^T,k^T,v^T [cin, tok] bf16) and
    weight slices, so contraction dims land on SBUF partitions directly.
  - scores are computed transposed ([ts, tq]): stationary k-tile [64,128],
    moving q [64,512].  exp on ScalarE (scale 1/8 folded), FD=1024.
  - PV runs TRANSPOSED: out^T[tq,d] accumulates over ts tiles with the exp
    tile as stationary [128,128] and v[ts,d] as moving [128,64] -- half the
    PE rows of the [65,512]-oriented PV.  A parallel 1-wide matmul against a
    ones vector accumulates the softmax denominator per tq ON PARTITIONS,
    so normalization is a per-partition reciprocal + tensor_scalar (no DMA
    broadcast round-trip).  PV consumes each exp tile the step after it is
    produced (self-interleaved), so no bulk PV drain remains at the end.
  - normalized att tiles [tq,256] are PE-transposed (identity matmul) back
    to [cout,tq] for the output projection, which is unchanged.
  - pair order (tb,hp) = (0,0),(1,0),(0,1),(1,1); drains are deferred into
    the next pair's phase-1 steps; the last pair's drain splits its PSUM
    evacuation copies between DVE and the (then idle) ScalarE.
  - q/k/v bias adds run on Pool (gpsimd) to keep DVE under the ScalarE
    roofline; ScalarE exp (~133 us) is the modeled bottleneck.
"""

import os
import numpy as np
import ml_dtypes

BF16 = ml_dtypes.bfloat16

B, TOKENS, C = 2, 2048, 1024
NHEAD, D = 16, 64
NCORES = 8
NGROUP = 4                # head groups (cores per batch)
COUT = C // NGROUP        # 256 head-channels per core
NH = NHEAD // NGROUP      # 4 heads per core

P = 128                   # SBUF partitions

# scores arrive in PSUM pre-scaled by 1/EXP_SCALE (folded into Wq/bq on the
# host): y = z/128 with z the scaled score.  ScalarE computes exp(y*128);
# DVE computes (1+y)^128 via 7 squarings (rel err ~ z^2/256, |z|<2.8 here).
EXP_SCALE = 1024.0        # host folds 1/1024 into Wq, bq
_DVE_EXP_NAME = "TENSOR_MASK"   # registry slot reused for the exp op


def _install_dve_exp():
    """Register the exp-by-squaring op in the custom-DVE registry (in-place
    swap of an existing slot; per-NEFF uop tables are generated from it)."""
    import numpy as np
    import concourse.dve_ops as dvo
    from concourse.dve_spec import Spec, Src0, C0, sq, lower as dve_lower
    from concourse.dve_uop import DveOpSpec

    cur = next(o for o in dvo.OPS if o.name == _DVE_EXP_NAME)
    if getattr(cur, "_is_exp128", False):
        return cur

    def ref(in0, in1, s0, s1, imm2):
        r = in0.astype(np.float32) + s0
        for _ in range(7):
            r = r * r
        return r

    body = Src0 + C0
    for _ in range(7):
        body = sq(body)
    spec = Spec(body=body, reference=ref)
    shas = {}
    for ver in ("v3", "v4"):
        uops = dve_lower(spec, ver=ver)
        shas[ver] = DveOpSpec(name=_DVE_EXP_NAME,
                              opcode=dvo.get_dve_sub_opcode(_DVE_EXP_NAME),
                              uops=uops, rd1_en=False).sha(ver)
    op = dvo.DveOp(name=_DVE_EXP_NAME, spec=spec, subdim=False,
                   uops_sha=shas)
    object.__setattr__(op, "_is_exp128", True)
    dvo._COMPILE_CACHE.pop((_DVE_EXP_NAME, "v3"), None)
    dvo._COMPILE_CACHE.pop((_DVE_EXP_NAME, "v4"), None)
    idx = next(i for i, o in enumerate(dvo.OPS) if o.name == _DVE_EXP_NAME)
    dvo.OPS[idx] = op
    return op


# exp tiles with (2*i+hh) % 16 in DVE_SLOTS run on DVE; rest on ScalarE.
DVE_SLOTS = (1, 4, 7, 10, 12, 15)


def build_nc(tok=TOKENS, cin=C, cout=COUT, nh=NH, reps=1, diag=None,
             dve_slots=None, bias_pool=False, evac_pool=False,
             s128=True, qk_bias_scalar=False, evac_alt=False,
             coarse_norm=True, wide=True, xcache=False):
    """Emit the per-core Bass module. d=64 fixed; cout = nh*64.
    reps>1 replicates the whole body (timing builds only): per-rep marginal
    time = steady-state kernel time with per-exec dispatch overhead removed.
    diag: timing-only work-gutting variants (output garbage):
      'noexp'    - exp activations shrunk to [128,32] (ScalarE ~0)
      'halfscore'- only half of each scores tile computed (PE -27us)
      'halfpv'   - pv_slot processes one head only (PE -14us)
      'halfproj' - m=1 q/k projection chunks skipped (PE -14us)"""
    import concourse.bacc as bacc
    import concourse.tile as tile
    import concourse.mybir as mybir
    from concourse import masks

    d = D
    assert cout == nh * d
    ncin = cin // P               # cin tiles (contraction)
    nt = tok // P                 # ts tiles
    nm = max(1, cout // P)        # 128-wide cout chunks (qhT/khT)
    tqb = min(1024, tok)          # tq block (exp FD)
    ntqb = tok // tqb
    ntb = tqb // P                # tq tiles per block (8)
    sck = min(512, tok)           # scores moving chunk / DMA chunk
    csk = tqb // sck
    w_m = 512                     # m0 proj matmul width (1 PSUM bank max)
    w_m2 = 512 if wide else 256   # m1 proj matmul width
    nko = max(1, cout // P)       # out-proj contraction tiles
    nob = max(1, cin // 256)      # out-proj 256-wide chunks

    fp32 = mybir.dt.float32
    bf16 = mybir.dt.bfloat16

    dve_exp_op = _install_dve_exp() if dve_slots else None

    nc = bacc.Bacc("TRN2", target_bir_lowering=False, debug=False)

    qT = nc.dram_tensor("qT", [cin, tok], bf16, kind="ExternalInput")
    kT = nc.dram_tensor("kT", [cin, tok], bf16, kind="ExternalInput")
    vT = nc.dram_tensor("vT", [cin, tok], bf16, kind="ExternalInput")
    wqT = nc.dram_tensor("wqT", [cin, cout], bf16, kind="ExternalInput")
    wkT = nc.dram_tensor("wkT", [cin, cout], bf16, kind="ExternalInput")
    wvT = nc.dram_tensor("wvT", [cin, cout], bf16, kind="ExternalInput")
    woT = nc.dram_tensor("woT", [cout, cin], bf16, kind="ExternalInput")
    bqv = nc.dram_tensor("bqv", [P, nm], fp32, kind="ExternalInput")
    bkv = nc.dram_tensor("bkv", [P, nm], fp32, kind="ExternalInput")
    bvv = nc.dram_tensor("bvv", [1, cout], fp32, kind="ExternalInput")
    outp = nc.dram_tensor("outp", [tok, cin], bf16, kind="ExternalOutput")

    with tile.TileContext(nc) as tc:
        from contextlib import ExitStack
        with ExitStack() as ctx:
            consts = ctx.enter_context(tc.tile_pool(name="consts", bufs=1))
            xstream = ctx.enter_context(tc.tile_pool(name="xstream", bufs=2))
            vstream = ctx.enter_context(tc.tile_pool(name="vstream", bufs=2))
            persist = ctx.enter_context(tc.tile_pool(name="persist", bufs=1))
            expool = ctx.enter_context(tc.tile_pool(name="expool", bufs=4))
            smalls = ctx.enter_context(tc.tile_pool(name="smalls", bufs=8))
            ostage = ctx.enter_context(tc.tile_pool(name="ostage", bufs=3))
            psum = ctx.enter_context(
                tc.tile_pool(name="psum", bufs=1, space="PSUM"))

            for _rep in range(reps):
                # ---- constants (K/Q m=0 weights first — gate the first exp)
                wq_sb = consts.tile([P, ncin, cout], bf16, tag="wq")
                wk_sb = consts.tile([P, ncin, cout], bf16, tag="wk")
                wv_sb = consts.tile([P, ncin, cout], bf16, tag="wv")
                wo_sb = consts.tile([P, nko, cin], bf16, tag="wo")
                bq_sb = consts.tile([P, nm], fp32, tag="bq")
                bk_sb = consts.tile([P, nm], fp32, tag="bk")
                bv_sb = consts.tile([P, nh, d], fp32, tag="bv")
                ones_sb = consts.tile([P, 1], bf16, tag="ones")

                for w_sb, w_h in ((wk_sb, wkT), (wq_sb, wqT)):
                    nc.scalar.dma_start(
                        out=w_sb[:, :, 0:P],
                        in_=w_h[:, 0:P].rearrange("(nb p) co -> p nb co", p=P))
                nc.sync.dma_start(out=bq_sb, in_=bqv[:, :])
                nc.sync.dma_start(out=bk_sb, in_=bkv[:, :])
                nc.vector.memset(ones_sb, 1.0)

                def emit_late_consts():
                    for w_sb, w_h in ((wk_sb, wkT), (wq_sb, wqT)):
                        if nm > 1:
                            nc.gpsimd.dma_start(
                                out=w_sb[:, :, P:cout],
                                in_=w_h[:, P:cout].rearrange(
                                    "(nb p) co -> p nb co", p=P))
                    nc.gpsimd.dma_start(
                        out=wv_sb,
                        in_=wvT[:, :].rearrange("(nb p) co -> p nb co", p=P))
                    nc.gpsimd.dma_start(
                        out=wo_sb,
                        in_=woT[:, :].rearrange("(nb p) co -> p nb co", p=P))
                    nc.gpsimd.dma_start(
                        out=bv_sb,
                        in_=bvv[:, :].rearrange("o (h e) -> o h e", h=nh)
                        .to_broadcast([P, nh, d]))

                # ---- persistent activations --------------------------------
                qh_sb = persist.tile([P, nm, tok], bf16, tag="qh")
                # kh: with s128, per-head 128-row slots (data rows h%2*64,
                # the other 64 rows zeroed) so scores contract over 128.
                if s128:
                    kh_sb = persist.tile([P, nh, tok], bf16, tag="kh")
                    for h in range(nh):
                        z0 = (1 - h % 2) * 64
                        nc.gpsimd.memset(kh_sb[z0:z0 + 64, h, :], 0.0)
                else:
                    kh_sb = persist.tile([P, nm, tok], bf16, tag="kh")
                vh_all = persist.tile([P, nt, nh, d], bf16, tag="vh")
                att_sb = persist.tile([P, ntqb, ntb, 2 * P], bf16, tag="attn")
                att_T = persist.tile([P, nko, tok], bf16, tag="attT")

                def bias_add(out, in0, b_ap):
                    if qk_bias_scalar:
                        nc.scalar.add(out=out, in_=in0, add=b_ap)
                    else:
                        nc.vector.tensor_scalar(
                            out=out, in0=in0, scalar1=b_ap, scalar2=None,
                            op0=mybir.AluOpType.add)

                def qk_evac(is_k, xh_sb, ps_slice, m, c0, cn, b_sb):
                    if is_k and s128:
                        for hh in range(2):
                            p0 = hh * 64
                            bias_add(
                                out=xh_sb[p0:p0 + 64, 2 * m + hh,
                                          c0:c0 + cn],
                                in0=ps_slice[p0:p0 + 64, :],
                                b_ap=b_sb[p0:p0 + 64, m:m + 1])
                    else:
                        bias_add(out=xh_sb[:, m, c0:c0 + cn],
                                 in0=ps_slice, b_ap=b_sb[:, m:m + 1])

                xt_cache = {}

                def emit_qk_chunk(x_h, w_sb, b_sb, xh_sb, it, m, xtag,
                                  use_s=False):
                    is_k = xtag == "xk"
                    key = (xtag, it)
                    if xcache and key in xt_cache:
                        xt = xt_cache[key]
                    else:
                        if xcache:
                            xt = persist.tile([P, ncin, sck], bf16,
                                              tag=f"xc_{xtag}_{it}")
                            xt_cache[key] = xt
                        else:
                            xt = xstream.tile([P, ncin, sck], bf16, tag=xtag,
                                              name=f"xt_{xtag}_{it}_{m}")
                        nc.sync.dma_start(
                            out=xt,
                            in_=x_h[:, :].rearrange("(nb p) t -> p nb t", p=P)
                            [:, :, it * sck:(it + 1) * sck])
                    if use_s:
                        # pre-phase: borrow the scores banks (idle until the
                        # first pair) so "pp" can stay single-buffered.
                        ps = psum.tile([P, tqb], fp32, tag="s", bufs=2,
                                       name="psqk")
                        for half in range(sck // w_m):
                            for ci in range(ncin):
                                nc.tensor.matmul(
                                    ps[:, half * w_m:(half + 1) * w_m],
                                    w_sb[:, ci, m * P:(m + 1) * P],
                                    xt[:, ci, half * w_m:(half + 1) * w_m],
                                    start=(ci == 0), stop=(ci == ncin - 1))
                        qk_evac(is_k, xh_sb, ps[:, 0:sck], m,
                                it * sck, sck, b_sb)
                        return
                    for half in range(sck // w_m2):
                        ps = psum.tile([P, w_m2], fp32, tag="pp", bufs=1,
                                       name="psqk")
                        for ci in range(ncin):
                            nc.tensor.matmul(
                                ps, w_sb[:, ci, m * P:(m + 1) * P],
                                xt[:, ci, half * w_m2:(half + 1) * w_m2],
                                start=(ci == 0), stop=(ci == ncin - 1))
                        c0 = it * sck + half * w_m2
                        qk_evac(is_k, xh_sb, ps, m, c0, w_m2, b_sb)

                def emit_v_tile(it):
                    vt = vstream.tile([P, ncin, P], bf16, tag="vt",
                                      name=f"vt_{it}")
                    nc.gpsimd.dma_start(
                        out=vt,
                        in_=vT[:, :].rearrange("(nb p) t -> p nb t", p=P)
                        [:, :, it * P:(it + 1) * P])
                    ps = psum.tile([P, cout], fp32, tag="pp", bufs=1,
                                   name="psv")
                    for ci in range(ncin):
                        nc.tensor.matmul(ps, vt[:, ci, :], wv_sb[:, ci, :],
                                         start=(ci == 0),
                                         stop=(ci == ncin - 1))
                    (nc.gpsimd if bias_pool else nc.vector).tensor_tensor(
                        out=vh_all[:, it, :, :],
                        in0=ps.rearrange("p (h e) -> p h e", h=nh),
                        in1=bv_sb,
                        op=mybir.AluOpType.add)

                # m=0 K/Q chunks upfront (kh m0 gates the first scores).
                for it in range(tok // sck):
                    emit_qk_chunk(kT, wk_sb, bk_sb, kh_sb, it, 0, "xk",
                                  use_s=True)
                    emit_qk_chunk(qT, wq_sb, bq_sb, qh_sb, it, 0, "xq",
                                  use_s=True)
                emit_late_consts()

                # ---- attention ---------------------------------------------
                # pair order: (tb,hp) -- m=1 projections stream during
                # pair (1,0); V tiles during pair (0,0).
                if ntqb > 1:
                    order = [(0, 0), (1, 0), (0, 1), (1, 1)]
                else:
                    order = [(0, 0), (0, 1)] if nh > 2 else [(0, 0)]
                m1_chunks = []
                if nm > 1 and diag != "halfproj":
                    m1_chunks = [(kT, wk_sb, bk_sb, kh_sb, it, 1, "xk")
                                 for it in range(tok // sck)]
                    m1_chunks += [(qT, wq_sb, bq_sb, qh_sb, it, 1, "xq")
                                  for it in range(tok // sck)]

                def copy_to(ceng, out, in_):
                    if ceng is nc.scalar:
                        nc.scalar.activation(
                            out=out, in_=in_,
                            func=mybir.ActivationFunctionType.Copy)
                    else:
                        ceng.tensor_copy(out=out, in_=in_)

                deferred = []

                def pop_deferred(k):
                    for _ in range(k):
                        if deferred:
                            deferred.pop(0)()

                exp_bufs = 16

                for pi, (tb, hp) in enumerate(order):
                    m = hp if nm > 1 else 0
                    heads = ((2 * hp, 0), (2 * hp + 1, d))
                    exs = {}
                    last = (pi == len(order) - 1)
                    pv = psum.tile([P, 2, ntb, d], fp32, tag="pv", bufs=1,
                                   name=f"pv_{tb}_{hp}")
                    den = psum.tile([P, 2 * ntb], fp32, tag="den", bufs=1,
                                    name=f"den_{tb}_{hp}")

                    def pv_slot(ts, pv=pv, den=den, heads=heads, exs=exs):
                        # one long-lived accumulation group per PSUM bank:
                        # bank hh holds 8 [128,64] regions (start pending-
                        # zeroes the whole 2KB bank, so only the bank's first
                        # matmul starts, only its last stops).
                        den_last_hh = 0 if diag == "halfpv" else 1
                        for hh, (h, p0) in enumerate(heads):
                            e = exs[(h, ts)]
                            for t in range(ntb):
                                if not (diag == "halfpv" and hh == 1):
                                    nc.tensor.matmul(
                                        pv[:, hh, t, :],
                                        e[:, t * P:(t + 1) * P],
                                        vh_all[:, ts, h, :],
                                        start=(ts == 0 and t == 0),
                                        stop=(ts == nt - 1 and t == ntb - 1),
                                        skip_group_check=not (
                                            (ts == 0 and t == 0)
                                            or (ts == nt - 1
                                                and t == ntb - 1)))
                                if diag == "noden":
                                    # keep one writer so den has a dep chain
                                    if ts == 0 and t == 0 and hh == 0:
                                        nc.tensor.matmul(
                                            den[:, 0:1],
                                            e[:, 0:P], ones_sb,
                                            start=True, stop=True)
                                    continue
                                col = hh * ntb + t
                                nc.tensor.matmul(
                                    den[:, col:col + 1],
                                    e[:, t * P:(t + 1) * P], ones_sb,
                                    start=(ts == 0 and hh == 0 and t == 0),
                                    stop=(ts == nt - 1 and hh == den_last_hh
                                          and t == ntb - 1),
                                    skip_group_check=not (
                                        (ts == 0 and hh == 0 and t == 0)
                                        or (ts == nt - 1 and hh == den_last_hh
                                            and t == ntb - 1)))

                    pv_next = 0
                    for i in range(nt):
                        for hh, (h, p0) in enumerate(heads):
                            s_ps = psum.tile([P, tqb], fp32, tag="s",
                                             bufs=2, name="s_ps")
                            ncc = 1 if diag == "halfscore" else csk
                            for cc in range(ncc):
                                q0 = tb * tqb + cc * sck
                                if s128:
                                    # 128-row contraction: kh slot h has the
                                    # other head's rows zeroed, so the packed
                                    # qh (both heads) contracts correctly.
                                    nc.tensor.matmul(
                                        s_ps[:, cc * sck:(cc + 1) * sck],
                                        kh_sb[:, h, i * P:(i + 1) * P],
                                        qh_sb[:, m, q0:q0 + sck],
                                        start=True, stop=True)
                                else:
                                    nc.tensor.matmul(
                                        s_ps[:, cc * sck:(cc + 1) * sck],
                                        kh_sb[p0:p0 + d, m,
                                              i * P:(i + 1) * P],
                                        qh_sb[p0:p0 + d, m, q0:q0 + sck],
                                        start=True, stop=True)
                            ex = expool.tile([P, tqb], bf16, tag="ex",
                                             bufs=exp_bufs,
                                             name=f"ex_{h}_{i}")
                            sc_scale = EXP_SCALE * float(d) ** -0.5
                            if diag == "noexp":
                                nc.scalar.activation(
                                    out=ex[:, 0:32], in_=s_ps[:, 0:32],
                                    func=mybir.ActivationFunctionType.Exp,
                                    scale=sc_scale)
                            elif (dve_slots
                                  and (2 * i + hh) % 16 in dve_slots):
                                nc.vector._custom_dve(
                                    dve_exp_op, out=ex, in0=s_ps, s0=1.0)
                            else:
                                nc.scalar.activation(
                                    out=ex, in_=s_ps,
                                    func=mybir.ActivationFunctionType.Exp,
                                    scale=sc_scale)
                            exs[(h, i)] = ex
                        if pi == 0:
                            emit_v_tile(i)
                        if pi == 1 and m1_chunks and i % 2 == 0:
                            emit_qk_chunk(*m1_chunks.pop(0))
                        budget = 2
                        while pv_next <= i - 1 and budget:
                            pv_slot(pv_next)
                            pv_next += 1
                            budget -= 1
                        pop_deferred(2 if last else 3)
                    while pv_next < nt:
                        pv_slot(pv_next)
                        pv_next += 1

                    # ---- drain: normalize (+ transpose/outproj when hp==1)
                    drain = []

                    def mk_norm(tb, hp, hh, h, t, pv=pv, den=den):
                        def go(ceng=nc.vector):
                            col = hh * ntb + t
                            rec = smalls.tile([P, 1], fp32, tag="rec",
                                              name=f"rec_{tb}_{h}_{t}")
                            nc.vector.reciprocal_approx_fast(
                                out=rec, in_=den[:, col:col + 1])
                            att_dst = att_sb[:, tb, t,
                                             (2 * hp + hh) * d:
                                             (2 * hp + hh + 1) * d]
                            if evac_alt and (t + hh) % 2:
                                nc.scalar.mul(out=att_dst,
                                              in_=pv[:, hh, t, :], mul=rec)
                            else:
                                nc.vector.tensor_scalar(
                                    out=att_dst,
                                    in0=pv[:, hh, t, :], scalar1=rec,
                                    scalar2=None, op0=mybir.AluOpType.mult)
                        return go

                    def mk_norm_coarse(tb, hp, hh, pv=pv, den=den):
                        # one recip + one broadcast-mult per (pair, head):
                        # [P,ntb] recips then [P,ntb,d] normalize.
                        def go(ceng=nc.vector):
                            rec8 = smalls.tile([P, ntb], fp32, tag="rec8",
                                               name=f"rec8_{tb}_{hp}_{hh}")
                            nc.vector.reciprocal_approx_fast(
                                out=rec8,
                                in_=den[:, hh * ntb:(hh + 1) * ntb])
                            nc.vector.tensor_tensor(
                                out=att_sb[:, tb, :,
                                           (2 * hp + hh) * d:
                                           (2 * hp + hh + 1) * d],
                                in0=pv[:, hh, :, :],
                                in1=rec8[:, :]
                                .rearrange("p (t o) -> p t o", o=1)
                                .to_broadcast([P, ntb, d]),
                                op=mybir.AluOpType.mult)
                        return go

                    def mk_transpose(tb, t):
                        def go(ceng=nc.vector):
                            tbase = tb * tqb + t * P
                            for ko in range(nko):
                                nc.sync.dma_start_transpose(
                                    out=att_T[:, ko, tbase:tbase + P],
                                    in_=att_sb[:, tb, t,
                                               ko * P:(ko + 1) * P])
                        return go

                    def mk_outproj(tb, t, tail=False):
                        def go(ceng=None):
                            if ceng is None:
                                ceng = (nc.scalar
                                        if (evac_alt and t % 2)
                                        else nc.vector)
                            tbase = tb * tqb + t * P
                            o_sb = ostage.tile([P, cin], bf16, tag="ost",
                                               name=f"ost_{tb}_{t}")
                            if tail:
                                # scores are done -- reuse their banks for
                                # [P,1024] chunks and one big evacuation copy.
                                for n2 in range(cin // tqb):
                                    ps = psum.tile([P, tqb], fp32, tag="s",
                                                   bufs=2, name="pso")
                                    for half in range(tqb // 512):
                                        for ko in range(nko):
                                            nc.tensor.matmul(
                                                ps[:, half * 512:
                                                   (half + 1) * 512],
                                                att_T[:, ko,
                                                      tbase:tbase + P],
                                                wo_sb[:, ko,
                                                      n2 * tqb + half * 512:
                                                      n2 * tqb
                                                      + (half + 1) * 512],
                                                start=(ko == 0),
                                                stop=(ko == nko - 1))
                                    copy_to(ceng,
                                            o_sb[:, n2 * tqb:(n2 + 1) * tqb],
                                            ps)
                            else:
                                for n in range(nob):
                                    ps = psum.tile([P, 256], fp32, tag="pp",
                                                   bufs=1, name="pso")
                                    for ko in range(nko):
                                        nc.tensor.matmul(
                                            ps,
                                            att_T[:, ko, tbase:tbase + P],
                                            wo_sb[:, ko,
                                                  n * 256:(n + 1) * 256],
                                            start=(ko == 0),
                                            stop=(ko == nko - 1))
                                    copy_to(ceng,
                                            o_sb[:, n * 256:(n + 1) * 256],
                                            ps)
                            nc.sync.dma_start(
                                out=outp[tbase:tbase + P, :], in_=o_sb)
                        go._alt = True
                        return go

                    if coarse_norm:
                        for hh in range(2):
                            drain.append(mk_norm_coarse(tb, hp, hh))
                        for t in range(ntb):
                            if hp == nh // 2 - 1:
                                drain.append(mk_transpose(tb, t))
                                drain.append(mk_outproj(tb, t, tail=last))
                    else:
                        for t in range(ntb):
                            for hh, (h, p0) in enumerate(heads):
                                drain.append(mk_norm(tb, hp, hh, h, t))
                            if hp == nh // 2 - 1:
                                drain.append(mk_transpose(tb, t))
                                drain.append(mk_outproj(tb, t, tail=last))

                    if not last:
                        deferred.extend(drain)
                    else:
                        # tail: alternate evacuation copies between DVE and
                        # the now-idle ScalarE.
                        pop_deferred(len(deferred))
                        oj = 0
                        for work in drain:
                            if getattr(work, "_alt", False):
                                work(ceng=(nc.scalar if oj % 2
                                           else nc.vector))
                                oj += 1
                            else:
                                work()

    nc.compile()
    return nc


def _host_inputs(q, k, v, Wq, Wk, Wv, Wo, bq, bk, bv,
                 tok=TOKENS, cin=C, cout=COUT, ngroup=NGROUP, ncores=NCORES):
    """Build per-core in_maps (host-side shard + transpose + bf16 cast)."""
    nm = max(1, cout // P)
    xT = {}
    for b in range(q.shape[0]):
        xT[('q', b)] = np.ascontiguousarray(q[b].T).astype(BF16)
        xT[('k', b)] = np.ascontiguousarray(k[b].T).astype(BF16)
        xT[('v', b)] = np.ascontiguousarray(v[b].T).astype(BF16)
    in_maps = []
    for core in range(ncores):
        b, g = core // ngroup, core % ngroup
        sl = slice(g * cout, (g + 1) * cout)
        in_maps.append({
            "qT": xT[('q', b)],
            "kT": xT[('k', b)],
            "vT": xT[('v', b)],
            "wqT": np.ascontiguousarray(
                Wq[sl, :].T / EXP_SCALE).astype(BF16),
            "wkT": np.ascontiguousarray(Wk[sl, :].T).astype(BF16),
            "wvT": np.ascontiguousarray(Wv[sl, :].T).astype(BF16),
            "woT": np.ascontiguousarray(Wo[:, sl].T).astype(BF16),
            "bqv": np.ascontiguousarray(
                bq[sl].reshape(nm, P).T / EXP_SCALE).astype(np.float32),
            "bkv": np.ascontiguousarray(
                bk[sl].reshape(nm, P).T).astype(np.float32),
            "bvv": np.ascontiguousarray(bv[sl][None, :]).astype(np.float32),
        })
    return in_maps


_NC_CACHE = {}


def _get_nc():
    if "nc" not in _NC_CACHE:
        _NC_CACHE["nc"] = build_nc()
    return _NC_CACHE["nc"]


def kernel(q, k, v, Wq, bq, Wk, bk, Wv, bv, Wo, bo):
    from concourse.bass_utils import run_bass_kernel_spmd

    q = np.asarray(q, dtype=np.float32)
    k = np.asarray(k, dtype=np.float32)
    v = np.asarray(v, dtype=np.float32)
    nc = _get_nc()
    in_maps = _host_inputs(q, k, v,
                           np.asarray(Wq, np.float32), np.asarray(Wk, np.float32),
                           np.asarray(Wv, np.float32), np.asarray(Wo, np.float32),
                           np.asarray(bq, np.float32), np.asarray(bk, np.float32),
                           np.asarray(bv, np.float32))
    res = run_bass_kernel_spmd(nc, in_maps, core_ids=list(range(NCORES)))
    parts = [np.asarray(r["outp"], dtype=np.float32) for r in res.results]
    out = np.stack(
        [sum(parts[b * NGROUP:(b + 1) * NGROUP]) for b in range(B)], axis=0)
    out = out + np.asarray(bo, np.float32)[None, None, :]
    return out.astype(np.float32)

